# revision 18
# baseline (speedup 1.0000x reference)
"""GAT layer (nn_GATLayer) as a Bass/Tile SPMD kernel on 8 trn2 NeuronCores.

Row-sharded: core c owns output rows [c*1024, (c+1)*1024).
  h = x @ W                       (local block + AllGather, fp16)
  e = leaky_relu(s_src[i] + s_dst[j]), s_* = h @ a_*
  masked = where(nbr>0, e, 0) == leaky_relu(nbr * (s_src[i]+s_dst[j]))
  att = softmax(masked, axis=1)   (no max-subtraction needed: |z| small)
  out = elu(att @ h)

Wall-clock (axon tunnel) optimizations vs v1:
  - adjacency shipped BITPACKED (u8, 32x fewer bytes; unpacked on DVE
    with shift+and per bit-plane)
  - x/w/att shipped as one fp16 tensor (fewer device_put round trips)
  - compute runs TRANSPOSED (partition=j, free=i): the attention matrix
    is produced directly in lhsT layout, so no PE transposes, no PSUM
    staging, no identity matrix. The bit-unpack column permutation
    (c = b*128+k <-> i = 8k+b) lands on the output-row side and is
    undone by a strided output DMA.
  - fp16 output, jax persistent compilation cache
"""

import sys

for _p in ("/opt/trn_rl_repo",):
    if _p not in sys.path:
        sys.path.insert(0, _p)

import numpy as np

N_CORES = 8
N = 8192               # nodes
D_IN = 512             # input features
D_OUT = 128            # output features
ROWS = N // N_CORES    # rows per core (1024)
N_IT = ROWS // 128     # i-subtiles per core (8)
N_JT = N // 128        # j-tiles (64)
HCOL = 132             # h row: 128 features + 1.0 + padding
KB = ROWS // 8         # packed mask bytes per row (128)
CW = 256               # combo width: [w | scale | pad] rows, att row
ALPHA = 0.2

_BUILT = {}


def _build_nc():
    import concourse.bacc as bacc
    import concourse.bass as bass
    import concourse.tile as tile
    from concourse import mybir

    f32 = mybir.dt.float32
    f16 = mybir.dt.float16
    u8 = mybir.dt.uint8
    i8 = mybir.dt.int8
    AF = mybir.ActivationFunctionType
    OP = mybir.AluOpType

    nc = bacc.Bacc("TRN2", target_bir_lowering=False, debug=False,
                   num_devices=N_CORES)
    DMA = nc.sync.dma_start

    # combo rows 0..511: [w | dequant scale | pad]; row 512: att (256 cols)
    combo_in = nc.declare_dram_parameter("combo", [D_IN + 1, CW], f16,
                                         isOutput=False)
    # x strip, int8-quantized per input feature d, d-major: xq[d, i]
    xq_in = nc.declare_dram_parameter("xq", [D_IN, ROWS], i8, isOutput=False)
    # maskp[j, k] bit b  =  (nbr[i_local=8k+b, j] > 0)
    mask_in = nc.declare_dram_parameter("maskp", [N, KB], u8, isOutput=False)
    out_d = nc.declare_dram_parameter("out", [ROWS, D_OUT], f16, isOutput=True)

    mask_r = mask_in[:, :].rearrange("(t p) k -> t p k", p=128)
    combo_ap = combo_in[:, :]
    out_ap = out_d[:, :]
    out_r = out_d[:, :].rearrange("(t p) n -> t p n", p=128)

    import os as _os
    if _os.environ.get("GAT_STOP"):
        with tile.TileContext(nc) as tc:
            with tc.tile_pool(name="sb", bufs=1) as sb:
                zt = sb.tile([128, D_OUT], f16)
                nc.vector.memset(zt, 0.0)
                for it in range(N_IT):
                    DMA(out=out_r[it], in_=zt)
        nc.compile()
        return nc

    with tile.TileContext(nc) as tc:
        with (
            tc.tile_pool(name="const", bufs=1) as const,
            tc.tile_pool(name="dram", bufs=1, space="DRAM") as dram,
            tc.tile_pool(name="mpool", bufs=3) as mpool,
            tc.tile_pool(name="zpool", bufs=3) as zpool,
            tc.tile_pool(name="ppool", bufs=3) as ppool,
            tc.tile_pool(name="sm", bufs=2) as sm,
        ):
            s_src_bc = const.tile([128, ROWS], f16)   # permuted: col b*KB+k -> i=8k+b
            sdc = const.tile([128, N_JT], f32)        # sdc[p, t] = s_dst[128t + p]
            h_aug = const.tile([128, N_JT, HCOL], f16)

            h16_loc = dram.tile([ROWS, HCOL], f16)
            h16_full = dram.tile([N, HCOL], f16)
            ssrc_loc = dram.tile([1, ROWS], f16)
            sd_loc = dram.tile([1, ROWS], f32)
            sd_full = dram.tile([1, N], f32)

            with (
                tc.tile_pool(name="pre_sb", bufs=1) as pre_sb,
                tc.tile_pool(name="pre_ps", bufs=2, space="PSUM") as pre_ps,
            ):
                att_row = pre_sb.tile([1, 2 * D_OUT], f16)
                DMA(out=att_row, in_=combo_in[D_IN:D_IN + 1, 0:2 * D_OUT])
                ones_1 = pre_sb.tile([1, 128], f16)
                nc.vector.memset(ones_1, 1.0)
                att_ps = pre_ps.tile([128, 2 * D_OUT], f32, tag="pp")
                nc.tensor.matmul(out=att_ps, lhsT=ones_1, rhs=att_row,
                                 start=True, stop=True)
                att_bc = pre_sb.tile([128, 2 * D_OUT], f32)
                nc.scalar.copy(out=att_bc, in_=att_ps)

                # w: combo[0:512, 0:128] -> [p, t, n], d_in = 128t + p
                w_sb = pre_sb.tile([128, 4, D_OUT], f16)
                DMA(out=w_sb,
                    in_=bass.AP(tensor=combo_ap.tensor, offset=0,
                                ap=[[CW, 128], [128 * CW, 4], [1, D_OUT]]))
                # dequant scales: combo[d, 128] -> [p, t], d = 128t + p
                sc16 = pre_sb.tile([128, 4], f16)
                DMA(out=sc16,
                    in_=bass.AP(tensor=combo_ap.tensor, offset=D_OUT,
                                ap=[[CW, 128], [128 * CW, 4], [1, 1]]))
                sc32 = pre_sb.tile([128, 4], f32)
                nc.vector.tensor_copy(out=sc32, in_=sc16)
                # x_t int8: xq[d, i] -> [p, t, s, q], d=128t+p, i=128s+q
                xt_i8 = pre_sb.tile([128, 4, N_IT, 128], i8)
                DMA(out=xt_i8,
                    in_=bass.AP(tensor=xq_in[:, :].tensor, offset=0,
                                ap=[[ROWS, 128], [128 * ROWS, 4], [128, N_IT],
                                    [1, 128]]))
                # dequant: xt16 = xq * scale[d]  (i8 -> f16, per-partition)
                xt_sb = pre_sb.tile([128, 4, N_IT, 128], f16)
                for t in range(4):
                    nc.vector.tensor_scalar(
                        out=xt_sb[:, t], in0=xt_i8[:, t],
                        scalar1=sc32[:, t:t + 1], scalar2=None, op0=OP.mult)

                h16_sb = pre_sb.tile([128, N_IT, HCOL], f16)
                nc.vector.memset(h16_sb[:, :, D_OUT:], 0.0)
                nc.gpsimd.memset(h16_sb[:, :, D_OUT:D_OUT + 1], 1.0)
                s_src_sb = pre_sb.tile([128, N_IT], f32)
                s_dst_sb = pre_sb.tile([128, N_IT], f32)
                scrap = pre_sb.tile([128, 128], f32)
                scrap2 = pre_sb.tile([128, 128], f32)
                for s in range(N_IT):
                    h_ps = pre_ps.tile([128, D_OUT], f32, tag="pp")
                    for t in range(4):
                        nc.tensor.matmul(out=h_ps, lhsT=xt_sb[:, t, s, :],
                                         rhs=w_sb[:, t, :],
                                         start=(t == 0), stop=(t == 3))
                    nc.vector.tensor_mul(scrap, h_ps, att_bc[:, :D_OUT])
                    nc.vector.tensor_reduce(
                        out=s_src_sb[:, s:s + 1], in_=scrap,
                        axis=mybir.AxisListType.X, op=OP.add)
                    nc.vector.tensor_mul(scrap2, h_ps, att_bc[:, D_OUT:])
                    nc.vector.tensor_reduce(
                        out=s_dst_sb[:, s:s + 1], in_=scrap2,
                        axis=mybir.AxisListType.X, op=OP.add)
                    nc.scalar.copy(out=h16_sb[:, s, :D_OUT], in_=h_ps)

                ssrc16 = pre_sb.tile([128, N_IT], f16)
                nc.vector.tensor_copy(out=ssrc16, in_=s_src_sb)

                # flatten to DRAM: flat[128s + p] = value[p, s]
                DMA(out=bass.AP(tensor=ssrc_loc[:, :].tensor, offset=0,
                                ap=[[1, 128], [128, N_IT]]),
                    in_=ssrc16)
                DMA(out=bass.AP(tensor=sd_loc[:, :].tensor, offset=0,
                                ap=[[1, 128], [128, N_IT]]),
                    in_=s_dst_sb)
                DMA(out=h16_loc[:, :].rearrange("(s p) c -> p s c", p=128),
                    in_=h16_sb)

                nc.gpsimd.collective_compute(
                    "AllGather", OP.bypass,
                    replica_groups=[list(range(N_CORES))],
                    ins=[h16_loc[:, :].opt()], outs=[h16_full[:, :].opt()])
                nc.gpsimd.collective_compute(
                    "AllGather", OP.bypass,
                    replica_groups=[list(range(N_CORES))],
                    ins=[sd_loc[:, :].opt()], outs=[sd_full[:, :].opt()])

                DMA(out=h_aug,
                    in_=h16_full[:, :].rearrange("(t p) c -> p t c", p=128))
                DMA(out=sdc,
                    in_=bass.AP(tensor=sd_full[:, :].tensor, offset=0,
                                ap=[[1, 128], [128, N_JT]]))
                # s_src broadcast, permuted: col b*KB+k -> s_src[8k+b]
                for b in range(8):
                    DMA(out=s_src_bc[:, b * KB:(b + 1) * KB],
                        in_=bass.AP(tensor=ssrc_loc[:, :].tensor, offset=b,
                                    ap=[[0, 128], [8, KB]]))

            # one PSUM bank per accumulator (a start=True matmul resets the
            # whole bank, so accumulator groups must not share banks);
            # opened only after pre_ps closes so all 8 banks are free
            hh_ps_cm = tc.tile_pool(name="hh_ps", bufs=1, space="PSUM")
            hh_ps = hh_ps_cm.__enter__()
            hh = []
            for m in range(N_IT):
                hh_m = hh_ps.tile([128, D_OUT + 1], f32, tag=f"hh{m}",
                                  name=f"hh{m}")
                hh.append(hh_m)

            # ---------------- main loop over j-tiles ----------------
            for jt in range(N_JT):
                p_u8 = mpool.tile([128, KB], u8, tag="pk")
                DMA(out=p_u8, in_=mask_r[jt])
                m8 = mpool.tile([128, ROWS], u8, tag="m8")
                for b in range(8):
                    nc.vector.tensor_scalar(
                        out=m8[:, b * KB:(b + 1) * KB], in0=p_u8,
                        scalar1=b, scalar2=1,
                        op0=OP.logical_shift_right, op1=OP.bitwise_and)
                z_t = zpool.tile([128, ROWS], f16, tag="z")
                nc.vector.scalar_tensor_tensor(
                    out=z_t, in0=s_src_bc, scalar=sdc[:, jt:jt + 1],
                    in1=m8, op0=OP.add, op1=OP.mult)
                nc.scalar.activation(out=z_t, in_=z_t, func=AF.Prelu,
                                     alpha=ALPHA)
                p_t = ppool.tile([128, ROWS], f16, tag="p")
                nc.scalar.activation(out=p_t, in_=z_t, func=AF.Exp)
                for m in range(N_IT):
                    nc.tensor.matmul(
                        out=hh[m], lhsT=p_t[:, m * 128:(m + 1) * 128],
                        rhs=h_aug[:, jt, :D_OUT + 1],
                        start=(jt == 0), stop=(jt == N_JT - 1))

            # ------------- epilogue: out = elu(hh[:, :128] / Z) -------------
            for m in range(N_IT):
                rz = sm.tile([128, 1], f32, tag="rz")
                nc.vector.reciprocal(out=rz, in_=hh[m][:, D_OUT:D_OUT + 1])
                tmin = sm.tile([128, D_OUT], f32, tag="tmin")
                nc.vector.tensor_scalar_min(tmin, hh[m][:, :D_OUT], 0.0)
                wmax = sm.tile([128, D_OUT], f32, tag="wmax")
                nc.vector.tensor_scalar(
                    out=wmax, in0=hh[m][:, :D_OUT], scalar1=0.0, scalar2=rz,
                    op0=OP.max, op1=OP.mult)
                e_t = sm.tile([128, D_OUT], f32, tag="et")
                nc.scalar.activation(out=e_t, in_=tmin, func=AF.Exp, scale=rz)
                o_t = sm.tile([128, D_OUT], f16, tag="ot")
                nc.vector.scalar_tensor_tensor(
                    out=o_t, in0=e_t, scalar=-1.0, in1=wmax,
                    op0=OP.add, op1=OP.add)
                # rows i = 8q + m  (undo the bit-plane permutation)
                DMA(out=bass.AP(tensor=out_ap.tensor, offset=D_OUT * m,
                                ap=[[8 * D_OUT, 128], [1, D_OUT]]),
                    in_=o_t)
            hh_ps_cm.__exit__(None, None, None)

    nc.compile()
    return nc


def _get_nc():
    if "nc" not in _BUILT:
        _BUILT["nc"] = _build_nc()
    return _BUILT["nc"]


_last_exec_ns = None


def _config_jax_cache():
    if "cache" in _BUILT:
        return
    _BUILT["cache"] = True
    try:
        import jax

        jax.config.update("jax_compilation_cache_dir", "/tmp/gat_jax_cache")
        jax.config.update("jax_persistent_cache_min_compile_time_secs", 0.0)
        jax.config.update("jax_persistent_cache_min_entry_size_bytes", 0)
    except Exception:
        pass


def kernel(x, immediate_neighbor, weights, attention):
    import os

    _config_jax_cache()
    from concourse.bass_utils import run_bass_kernel_spmd

    x = np.asarray(x, dtype=np.float32)
    nbr = np.asarray(immediate_neighbor)
    w = np.asarray(weights, dtype=np.float32)
    att = np.asarray(attention, dtype=np.float32).reshape(2 * D_OUT)

    # x -> int8 with per-feature scales (halves the incompressible wire
    # bytes; adds ~5e-3 rel err, well inside the 2e-2 gate)
    amax = np.maximum(np.abs(x).max(axis=0), 1e-30)
    inv = (127.0 / amax).astype(np.float32)
    xq = np.rint(x * inv[None, :]).astype(np.int8)   # [N, D_IN]
    xqT = np.ascontiguousarray(xq.T)                 # [D_IN, N]

    combo = np.empty((N_CORES, D_IN + 1, CW), np.float16)
    combo[:, :D_IN, :D_OUT] = w
    combo[:, :D_IN, D_OUT] = (amax / 127.0).astype(np.float16)
    combo[:, :D_IN, D_OUT + 1:] = 0
    combo[:, D_IN, :2 * D_OUT] = att

    # prepack[k, j] bit b = (nbr[8k+b, j] > 0)  (== packbits(nbr > 0,
    # axis=0, bitorder='little'), via strided shift/OR: ~5x faster).
    # Column-chunked so the accumulator and temps stay in cache.
    prepack = np.empty((N // 8, N), np.uint8)
    _tmp = np.empty((N // 8, 2048), np.uint8)
    _tb = _tmp.view(bool)
    for j0 in range(0, N, 2048):
        acc = prepack[:, j0:j0 + 2048]
        np.greater(nbr[0::8, j0:j0 + 2048], 0, out=acc.view(bool))
        for b in range(1, 8):
            np.greater(nbr[b::8, j0:j0 + 2048], 0, out=_tb)
            np.left_shift(_tmp, b, out=_tmp)
            np.bitwise_or(acc, _tmp, out=acc)

    nc = _get_nc()
    in_maps = []
    for c in range(N_CORES):
        in_maps.append({
            "combo": combo[c],
            "xq": xqT[:, c * ROWS:(c + 1) * ROWS],
            # transposed view: run_bass_via_pjrt's np.concatenate does the
            # single gather copy, so no ascontiguousarray here
            "maskp": prepack[c * KB:(c + 1) * KB, :].T,
        })
    kw = {}
    if os.environ.get("GAT_TRACE"):
        kw["trace"] = True
        tdir = os.environ.get("GAT_TRACE_DIR", "/tmp/gat_trace")
        os.makedirs(tdir, exist_ok=True)
        kw["tmpdir"] = tdir
    res = run_bass_kernel_spmd(nc, in_maps, list(range(N_CORES)), **kw)
    global _last_exec_ns
    _last_exec_ns = res.exec_time_ns
    out = np.empty((N, D_OUT), np.float32)
    for c in range(N_CORES):
        out[c * ROWS:(c + 1) * ROWS] = res.results[c]["out"]
    return out


# revision 19
# speedup vs baseline: 1.0499x; 1.0499x over previous
"""GAT layer (nn_GATLayer) as a Bass/Tile SPMD kernel on 8 trn2 NeuronCores.

Row-sharded: core c owns output rows [c*1024, (c+1)*1024).
  h = x @ W                       (local block + AllGather, fp16)
  e = leaky_relu(s_src[i] + s_dst[j]), s_* = h @ a_*
  masked = where(nbr>0, e, 0) == leaky_relu(nbr * (s_src[i]+s_dst[j]))
  att = softmax(masked, axis=1)   (no max-subtraction needed: |z| small)
  out = elu(att @ h)

Wall-clock (axon tunnel) optimizations vs v1:
  - adjacency shipped BITPACKED (u8, 32x fewer bytes; unpacked on DVE
    with shift+and per bit-plane)
  - x/w/att shipped as one fp16 tensor (fewer device_put round trips)
  - compute runs TRANSPOSED (partition=j, free=i): the attention matrix
    is produced directly in lhsT layout, so no PE transposes, no PSUM
    staging, no identity matrix. The bit-unpack column permutation
    (c = b*128+k <-> i = 8k+b) lands on the output-row side and is
    undone by a strided output DMA.
  - fp16 output, jax persistent compilation cache
"""

import sys

for _p in ("/opt/trn_rl_repo",):
    if _p not in sys.path:
        sys.path.insert(0, _p)

import numpy as np

N_CORES = 8
N = 8192               # nodes
D_IN = 512             # input features
D_OUT = 128            # output features
ROWS = N // N_CORES    # rows per core (1024)
N_IT = ROWS // 128     # i-subtiles per core (8)
N_JT = N // 128        # j-tiles (64)
HCOL = 132             # h row: 128 features + 1.0 + padding
KB = ROWS // 8         # packed mask bytes per row (128)
CW = 1152              # combo width: 1024 (x_t) + 128 (w)
ALPHA = 0.2

_BUILT = {}


def _build_nc():
    import concourse.bacc as bacc
    import concourse.bass as bass
    import concourse.tile as tile
    from concourse import mybir

    f32 = mybir.dt.float32
    f16 = mybir.dt.float16
    u8 = mybir.dt.uint8
    AF = mybir.ActivationFunctionType
    OP = mybir.AluOpType

    nc = bacc.Bacc("TRN2", target_bir_lowering=False, debug=False,
                   num_devices=N_CORES)
    DMA = nc.sync.dma_start

    # combo rows 0..511: [x_t strip | w]; row 512: att (256 cols)
    combo_in = nc.declare_dram_parameter("combo", [D_IN + 1, CW], f16,
                                         isOutput=False)
    # maskp[j, k] bit b  =  (nbr[i_local=8k+b, j] > 0)
    mask_in = nc.declare_dram_parameter("maskp", [N, KB], u8, isOutput=False)
    out_d = nc.declare_dram_parameter("out", [ROWS, D_OUT], f16, isOutput=True)

    mask_r = mask_in[:, :].rearrange("(t p) k -> t p k", p=128)
    combo_ap = combo_in[:, :]
    out_ap = out_d[:, :]
    out_r = out_d[:, :].rearrange("(t p) n -> t p n", p=128)

    import os as _os
    if _os.environ.get("GAT_STOP"):
        with tile.TileContext(nc) as tc:
            with tc.tile_pool(name="sb", bufs=1) as sb:
                zt = sb.tile([128, D_OUT], f16)
                nc.vector.memset(zt, 0.0)
                for it in range(N_IT):
                    DMA(out=out_r[it], in_=zt)
        nc.compile()
        return nc

    with tile.TileContext(nc) as tc:
        with (
            tc.tile_pool(name="const", bufs=1) as const,
            tc.tile_pool(name="dram", bufs=1, space="DRAM") as dram,
            tc.tile_pool(name="mpool", bufs=3) as mpool,
            tc.tile_pool(name="zpool", bufs=3) as zpool,
            tc.tile_pool(name="ppool", bufs=3) as ppool,
            tc.tile_pool(name="sm", bufs=2) as sm,
        ):
            s_src_bc = const.tile([128, ROWS], f16)   # permuted: col b*KB+k -> i=8k+b
            sdc = const.tile([128, N_JT], f32)        # sdc[p, t] = s_dst[128t + p]
            h_aug = const.tile([128, N_JT, HCOL], f16)

            h16_loc = dram.tile([ROWS, HCOL], f16)
            h16_full = dram.tile([N, HCOL], f16)
            ssrc_loc = dram.tile([1, ROWS], f16)
            sd_loc = dram.tile([1, ROWS], f32)
            sd_full = dram.tile([1, N], f32)

            with (
                tc.tile_pool(name="pre_sb", bufs=1) as pre_sb,
                tc.tile_pool(name="pre_ps", bufs=2, space="PSUM") as pre_ps,
            ):
                att_row = pre_sb.tile([1, 2 * D_OUT], f16)
                DMA(out=att_row, in_=combo_in[D_IN:D_IN + 1, 0:2 * D_OUT])
                ones_1 = pre_sb.tile([1, 128], f16)
                nc.vector.memset(ones_1, 1.0)
                att_ps = pre_ps.tile([128, 2 * D_OUT], f32, tag="pp")
                nc.tensor.matmul(out=att_ps, lhsT=ones_1, rhs=att_row,
                                 start=True, stop=True)
                att_bc = pre_sb.tile([128, 2 * D_OUT], f32)
                nc.scalar.copy(out=att_bc, in_=att_ps)

                # w: combo[0:512, 1024:1152] -> [p, t, n], d_in = 128t + p
                w_sb = pre_sb.tile([128, 4, D_OUT], f16)
                DMA(out=w_sb,
                    in_=bass.AP(tensor=combo_ap.tensor, offset=1024,
                                ap=[[CW, 128], [128 * CW, 4], [1, D_OUT]]))
                # x_t: combo[0:512, 0:1024] -> [p, t, s, q], d=128t+p, i=128s+q
                xt_sb = pre_sb.tile([128, 4, N_IT, 128], f16)
                DMA(out=xt_sb,
                    in_=bass.AP(tensor=combo_ap.tensor, offset=0,
                                ap=[[CW, 128], [128 * CW, 4], [128, N_IT],
                                    [1, 128]]))

                h16_sb = pre_sb.tile([128, N_IT, HCOL], f16)
                nc.vector.memset(h16_sb[:, :, D_OUT:], 0.0)
                nc.gpsimd.memset(h16_sb[:, :, D_OUT:D_OUT + 1], 1.0)
                s_src_sb = pre_sb.tile([128, N_IT], f32)
                s_dst_sb = pre_sb.tile([128, N_IT], f32)
                scrap = pre_sb.tile([128, 128], f32)
                scrap2 = pre_sb.tile([128, 128], f32)
                for s in range(N_IT):
                    h_ps = pre_ps.tile([128, D_OUT], f32, tag="pp")
                    for t in range(4):
                        nc.tensor.matmul(out=h_ps, lhsT=xt_sb[:, t, s, :],
                                         rhs=w_sb[:, t, :],
                                         start=(t == 0), stop=(t == 3))
                    nc.vector.tensor_mul(scrap, h_ps, att_bc[:, :D_OUT])
                    nc.vector.tensor_reduce(
                        out=s_src_sb[:, s:s + 1], in_=scrap,
                        axis=mybir.AxisListType.X, op=OP.add)
                    nc.vector.tensor_mul(scrap2, h_ps, att_bc[:, D_OUT:])
                    nc.vector.tensor_reduce(
                        out=s_dst_sb[:, s:s + 1], in_=scrap2,
                        axis=mybir.AxisListType.X, op=OP.add)
                    nc.scalar.copy(out=h16_sb[:, s, :D_OUT], in_=h_ps)

                ssrc16 = pre_sb.tile([128, N_IT], f16)
                nc.vector.tensor_copy(out=ssrc16, in_=s_src_sb)

                # flatten to DRAM: flat[128s + p] = value[p, s]
                DMA(out=bass.AP(tensor=ssrc_loc[:, :].tensor, offset=0,
                                ap=[[1, 128], [128, N_IT]]),
                    in_=ssrc16)
                DMA(out=bass.AP(tensor=sd_loc[:, :].tensor, offset=0,
                                ap=[[1, 128], [128, N_IT]]),
                    in_=s_dst_sb)
                DMA(out=h16_loc[:, :].rearrange("(s p) c -> p s c", p=128),
                    in_=h16_sb)

                nc.gpsimd.collective_compute(
                    "AllGather", OP.bypass,
                    replica_groups=[list(range(N_CORES))],
                    ins=[h16_loc[:, :].opt()], outs=[h16_full[:, :].opt()])
                nc.gpsimd.collective_compute(
                    "AllGather", OP.bypass,
                    replica_groups=[list(range(N_CORES))],
                    ins=[sd_loc[:, :].opt()], outs=[sd_full[:, :].opt()])

                DMA(out=h_aug,
                    in_=h16_full[:, :].rearrange("(t p) c -> p t c", p=128))
                DMA(out=sdc,
                    in_=bass.AP(tensor=sd_full[:, :].tensor, offset=0,
                                ap=[[1, 128], [128, N_JT]]))
                # s_src broadcast, permuted: col b*KB+k -> s_src[8k+b]
                for b in range(8):
                    DMA(out=s_src_bc[:, b * KB:(b + 1) * KB],
                        in_=bass.AP(tensor=ssrc_loc[:, :].tensor, offset=b,
                                    ap=[[0, 128], [8, KB]]))

            # one PSUM bank per accumulator (a start=True matmul resets the
            # whole bank, so accumulator groups must not share banks);
            # opened only after pre_ps closes so all 8 banks are free
            hh_ps_cm = tc.tile_pool(name="hh_ps", bufs=1, space="PSUM")
            hh_ps = hh_ps_cm.__enter__()
            hh = []
            for m in range(N_IT):
                hh_m = hh_ps.tile([128, D_OUT + 1], f32, tag=f"hh{m}",
                                  name=f"hh{m}")
                hh.append(hh_m)

            # ---------------- main loop over j-tiles ----------------
            for jt in range(N_JT):
                p_u8 = mpool.tile([128, KB], u8, tag="pk")
                DMA(out=p_u8, in_=mask_r[jt])
                m8 = mpool.tile([128, ROWS], u8, tag="m8")
                for b in range(8):
                    nc.vector.tensor_scalar(
                        out=m8[:, b * KB:(b + 1) * KB], in0=p_u8,
                        scalar1=b, scalar2=1,
                        op0=OP.logical_shift_right, op1=OP.bitwise_and)
                z_t = zpool.tile([128, ROWS], f16, tag="z")
                nc.vector.scalar_tensor_tensor(
                    out=z_t, in0=s_src_bc, scalar=sdc[:, jt:jt + 1],
                    in1=m8, op0=OP.add, op1=OP.mult)
                nc.scalar.activation(out=z_t, in_=z_t, func=AF.Prelu,
                                     alpha=ALPHA)
                p_t = ppool.tile([128, ROWS], f16, tag="p")
                nc.scalar.activation(out=p_t, in_=z_t, func=AF.Exp)
                for m in range(N_IT):
                    nc.tensor.matmul(
                        out=hh[m], lhsT=p_t[:, m * 128:(m + 1) * 128],
                        rhs=h_aug[:, jt, :D_OUT + 1],
                        start=(jt == 0), stop=(jt == N_JT - 1))

            # ------------- epilogue: out = elu(hh[:, :128] / Z) -------------
            for m in range(N_IT):
                rz = sm.tile([128, 1], f32, tag="rz")
                nc.vector.reciprocal(out=rz, in_=hh[m][:, D_OUT:D_OUT + 1])
                tmin = sm.tile([128, D_OUT], f32, tag="tmin")
                nc.vector.tensor_scalar_min(tmin, hh[m][:, :D_OUT], 0.0)
                wmax = sm.tile([128, D_OUT], f32, tag="wmax")
                nc.vector.tensor_scalar(
                    out=wmax, in0=hh[m][:, :D_OUT], scalar1=0.0, scalar2=rz,
                    op0=OP.max, op1=OP.mult)
                e_t = sm.tile([128, D_OUT], f32, tag="et")
                nc.scalar.activation(out=e_t, in_=tmin, func=AF.Exp, scale=rz)
                o_t = sm.tile([128, D_OUT], f16, tag="ot")
                nc.vector.scalar_tensor_tensor(
                    out=o_t, in0=e_t, scalar=-1.0, in1=wmax,
                    op0=OP.add, op1=OP.add)
                # rows i = 8q + m  (undo the bit-plane permutation)
                DMA(out=bass.AP(tensor=out_ap.tensor, offset=D_OUT * m,
                                ap=[[8 * D_OUT, 128], [1, D_OUT]]),
                    in_=o_t)
            hh_ps_cm.__exit__(None, None, None)

    nc.compile()
    return nc


def _get_nc():
    if "nc" not in _BUILT:
        _BUILT["nc"] = _build_nc()
    return _BUILT["nc"]


_last_exec_ns = None


def _config_jax_cache():
    if "cache" in _BUILT:
        return
    _BUILT["cache"] = True
    try:
        import jax

        jax.config.update("jax_compilation_cache_dir", "/tmp/gat_jax_cache")
        jax.config.update("jax_persistent_cache_min_compile_time_secs", 0.0)
        jax.config.update("jax_persistent_cache_min_entry_size_bytes", 0)
    except Exception:
        pass


def kernel(x, immediate_neighbor, weights, attention):
    import os

    _config_jax_cache()
    from concourse.bass_utils import run_bass_kernel_spmd

    x = np.asarray(x, dtype=np.float32)
    nbr = np.asarray(immediate_neighbor)
    w = np.asarray(weights, dtype=np.float32)
    att = np.asarray(attention, dtype=np.float32).reshape(2 * D_OUT)

    combo = np.empty((N_CORES, D_IN + 1, CW), np.float16)
    combo[:, :D_IN, :ROWS] = x.reshape(N_CORES, ROWS, D_IN).transpose(0, 2, 1)
    combo[:, :D_IN, ROWS:] = w
    combo[:, D_IN, :2 * D_OUT] = att
    combo[:, D_IN, 2 * D_OUT:] = 0  # device never reads this tail; keep it
    # zeroed anyway so the transfer stays compressible

    # prepack[k, j] bit b = (nbr[8k+b, j] > 0)  (== packbits(nbr > 0,
    # axis=0, bitorder='little'), via strided shift/OR: ~5x faster).
    # Column-chunked so the accumulator and temps stay in cache.
    prepack = np.empty((N // 8, N), np.uint8)
    _tmp = np.empty((N // 8, 2048), np.uint8)
    _tb = _tmp.view(bool)
    for j0 in range(0, N, 2048):
        acc = prepack[:, j0:j0 + 2048]
        np.greater(nbr[0::8, j0:j0 + 2048], 0, out=acc.view(bool))
        for b in range(1, 8):
            np.greater(nbr[b::8, j0:j0 + 2048], 0, out=_tb)
            np.left_shift(_tmp, b, out=_tmp)
            np.bitwise_or(acc, _tmp, out=acc)

    nc = _get_nc()
    in_maps = []
    for c in range(N_CORES):
        in_maps.append({
            "combo": combo[c],
            # transposed view: run_bass_via_pjrt's np.concatenate does the
            # single gather copy, so no ascontiguousarray here
            "maskp": prepack[c * KB:(c + 1) * KB, :].T,
        })
    kw = {}
    if os.environ.get("GAT_TRACE"):
        kw["trace"] = True
        tdir = os.environ.get("GAT_TRACE_DIR", "/tmp/gat_trace")
        os.makedirs(tdir, exist_ok=True)
        kw["tmpdir"] = tdir
    res = run_bass_kernel_spmd(nc, in_maps, list(range(N_CORES)), **kw)
    global _last_exec_ns
    _last_exec_ns = res.exec_time_ns
    out = np.empty((N, D_OUT), np.float32)
    for c in range(N_CORES):
        out[c * ROWS:(c + 1) * ROWS] = res.results[c]["out"]
    return out


# revision 21
# speedup vs baseline: 1.1588x; 1.1037x over previous
"""GAT layer (nn_GATLayer) as a Bass/Tile SPMD kernel on 8 trn2 NeuronCores.

Row-sharded: core c owns output rows [c*1024, (c+1)*1024).
  h = x @ W                       (local block + AllGather, fp16)
  e = leaky_relu(s_src[i] + s_dst[j]), s_* = h @ a_*
  masked = where(nbr>0, e, 0) == leaky_relu(nbr * (s_src[i]+s_dst[j]))
  att = softmax(masked, axis=1)   (no max-subtraction needed: |z| small)
  out = elu(att @ h)

Wall-clock (axon tunnel) optimizations vs v1:
  - adjacency shipped BITPACKED (u8, 32x fewer bytes; unpacked on DVE
    with shift+and per bit-plane)
  - x/w/att shipped as one fp16 tensor (fewer device_put round trips)
  - compute runs TRANSPOSED (partition=j, free=i): the attention matrix
    is produced directly in lhsT layout, so no PE transposes, no PSUM
    staging, no identity matrix. The bit-unpack column permutation
    (c = b*128+k <-> i = 8k+b) lands on the output-row side and is
    undone by a strided output DMA.
  - fp16 output, jax persistent compilation cache
"""

import sys

for _p in ("/opt/trn_rl_repo",):
    if _p not in sys.path:
        sys.path.insert(0, _p)

import numpy as np

N_CORES = 8
N = 8192               # nodes
D_IN = 512             # input features
D_OUT = 128            # output features
ROWS = N // N_CORES    # rows per core (1024)
N_IT = ROWS // 128     # i-subtiles per core (8)
N_JT = N // 128        # j-tiles (64)
HCOL = 132             # h row: 128 features + 1.0 + padding
KB = ROWS // 8         # packed mask bytes per row (128)
CW = 1152              # combo width: 1024 (x_t) + 128 (w)
ALPHA = 0.2

_BUILT = {}


def _build_nc():
    import concourse.bacc as bacc
    import concourse.bass as bass
    import concourse.tile as tile
    from concourse import mybir

    f32 = mybir.dt.float32
    f16 = mybir.dt.float16
    u8 = mybir.dt.uint8
    AF = mybir.ActivationFunctionType
    OP = mybir.AluOpType

    nc = bacc.Bacc("TRN2", target_bir_lowering=False, debug=False,
                   num_devices=N_CORES)
    DMA = nc.sync.dma_start

    # combo rows 0..511: [x_t strip | w]; row 512: att (256 cols)
    combo_in = nc.declare_dram_parameter("combo", [D_IN + 1, CW], f16,
                                         isOutput=False)
    # maskp[j, k] bit b  =  (nbr[i_local=8k+b, j] > 0)
    mask_in = nc.declare_dram_parameter("maskp", [N, KB], u8, isOutput=False)
    out_d = nc.declare_dram_parameter("out", [ROWS, D_OUT], f16, isOutput=True)

    mask_r = mask_in[:, :].rearrange("(t p) k -> t p k", p=128)
    combo_ap = combo_in[:, :]
    out_ap = out_d[:, :]
    out_r = out_d[:, :].rearrange("(t p) n -> t p n", p=128)

    import os as _os
    if _os.environ.get("GAT_STOP"):
        with tile.TileContext(nc) as tc:
            with tc.tile_pool(name="sb", bufs=1) as sb:
                zt = sb.tile([128, D_OUT], f16)
                nc.vector.memset(zt, 0.0)
                for it in range(N_IT):
                    DMA(out=out_r[it], in_=zt)
        nc.compile()
        return nc

    with tile.TileContext(nc) as tc:
        with (
            tc.tile_pool(name="const", bufs=1) as const,
            tc.tile_pool(name="dram", bufs=1, space="DRAM") as dram,
            tc.tile_pool(name="mpool", bufs=3) as mpool,
            tc.tile_pool(name="zpool", bufs=3) as zpool,
            tc.tile_pool(name="ppool", bufs=3) as ppool,
            tc.tile_pool(name="sm", bufs=2) as sm,
        ):
            s_src_bc = const.tile([128, ROWS], f16)   # permuted: col b*KB+k -> i=8k+b
            sdc = const.tile([128, N_JT], f32)        # sdc[p, t] = s_dst[128t + p]
            h_aug = const.tile([128, N_JT, HCOL], f16)

            h16_loc = dram.tile([ROWS, HCOL], f16)
            h16_full = dram.tile([N, HCOL], f16)
            ssrc_loc = dram.tile([1, ROWS], f16)
            sd_loc = dram.tile([1, ROWS], f32)
            sd_full = dram.tile([1, N], f32)

            with (
                tc.tile_pool(name="pre_sb", bufs=1) as pre_sb,
                tc.tile_pool(name="pre_ps", bufs=2, space="PSUM") as pre_ps,
            ):
                att_row = pre_sb.tile([1, 2 * D_OUT], f16)
                DMA(out=att_row, in_=combo_in[D_IN:D_IN + 1, 0:2 * D_OUT])
                ones_1 = pre_sb.tile([1, 128], f16)
                nc.vector.memset(ones_1, 1.0)
                att_ps = pre_ps.tile([128, 2 * D_OUT], f32, tag="pp")
                nc.tensor.matmul(out=att_ps, lhsT=ones_1, rhs=att_row,
                                 start=True, stop=True)
                att_bc = pre_sb.tile([128, 2 * D_OUT], f32)
                nc.scalar.copy(out=att_bc, in_=att_ps)

                # w: combo[0:512, 1024:1152] -> [p, t, n], d_in = 128t + p
                w_sb = pre_sb.tile([128, 4, D_OUT], f16)
                DMA(out=w_sb,
                    in_=bass.AP(tensor=combo_ap.tensor, offset=1024,
                                ap=[[CW, 128], [128 * CW, 4], [1, D_OUT]]))
                # x_t: combo[0:512, 0:1024] -> [p, t, s, q], d=128t+p, i=128s+q
                xt_sb = pre_sb.tile([128, 4, N_IT, 128], f16)
                DMA(out=xt_sb,
                    in_=bass.AP(tensor=combo_ap.tensor, offset=0,
                                ap=[[CW, 128], [128 * CW, 4], [128, N_IT],
                                    [1, 128]]))

                h16_sb = pre_sb.tile([128, N_IT, HCOL], f16)
                nc.vector.memset(h16_sb[:, :, D_OUT:], 0.0)
                nc.gpsimd.memset(h16_sb[:, :, D_OUT:D_OUT + 1], 1.0)
                s_src_sb = pre_sb.tile([128, N_IT], f32)
                s_dst_sb = pre_sb.tile([128, N_IT], f32)
                scrap = pre_sb.tile([128, 128], f32)
                scrap2 = pre_sb.tile([128, 128], f32)
                for s in range(N_IT):
                    h_ps = pre_ps.tile([128, D_OUT], f32, tag="pp")
                    for t in range(4):
                        nc.tensor.matmul(out=h_ps, lhsT=xt_sb[:, t, s, :],
                                         rhs=w_sb[:, t, :],
                                         start=(t == 0), stop=(t == 3))
                    nc.vector.tensor_mul(scrap, h_ps, att_bc[:, :D_OUT])
                    nc.vector.tensor_reduce(
                        out=s_src_sb[:, s:s + 1], in_=scrap,
                        axis=mybir.AxisListType.X, op=OP.add)
                    nc.vector.tensor_mul(scrap2, h_ps, att_bc[:, D_OUT:])
                    nc.vector.tensor_reduce(
                        out=s_dst_sb[:, s:s + 1], in_=scrap2,
                        axis=mybir.AxisListType.X, op=OP.add)
                    nc.scalar.copy(out=h16_sb[:, s, :D_OUT], in_=h_ps)

                ssrc16 = pre_sb.tile([128, N_IT], f16)
                nc.vector.tensor_copy(out=ssrc16, in_=s_src_sb)

                # flatten to DRAM: flat[128s + p] = value[p, s]
                DMA(out=bass.AP(tensor=ssrc_loc[:, :].tensor, offset=0,
                                ap=[[1, 128], [128, N_IT]]),
                    in_=ssrc16)
                DMA(out=bass.AP(tensor=sd_loc[:, :].tensor, offset=0,
                                ap=[[1, 128], [128, N_IT]]),
                    in_=s_dst_sb)
                DMA(out=h16_loc[:, :].rearrange("(s p) c -> p s c", p=128),
                    in_=h16_sb)

                nc.gpsimd.collective_compute(
                    "AllGather", OP.bypass,
                    replica_groups=[list(range(N_CORES))],
                    ins=[h16_loc[:, :].opt()], outs=[h16_full[:, :].opt()])
                nc.gpsimd.collective_compute(
                    "AllGather", OP.bypass,
                    replica_groups=[list(range(N_CORES))],
                    ins=[sd_loc[:, :].opt()], outs=[sd_full[:, :].opt()])

                DMA(out=h_aug,
                    in_=h16_full[:, :].rearrange("(t p) c -> p t c", p=128))
                DMA(out=sdc,
                    in_=bass.AP(tensor=sd_full[:, :].tensor, offset=0,
                                ap=[[1, 128], [128, N_JT]]))
                # s_src broadcast, permuted: col b*KB+k -> s_src[8k+b]
                for b in range(8):
                    DMA(out=s_src_bc[:, b * KB:(b + 1) * KB],
                        in_=bass.AP(tensor=ssrc_loc[:, :].tensor, offset=b,
                                    ap=[[0, 128], [8, KB]]))

            # one PSUM bank per accumulator (a start=True matmul resets the
            # whole bank, so accumulator groups must not share banks);
            # opened only after pre_ps closes so all 8 banks are free
            hh_ps_cm = tc.tile_pool(name="hh_ps", bufs=1, space="PSUM")
            hh_ps = hh_ps_cm.__enter__()
            hh = []
            for m in range(N_IT):
                hh_m = hh_ps.tile([128, D_OUT + 1], f32, tag=f"hh{m}",
                                  name=f"hh{m}")
                hh.append(hh_m)

            # ---------------- main loop over j-tiles ----------------
            for jt in range(N_JT):
                p_u8 = mpool.tile([128, KB], u8, tag="pk")
                DMA(out=p_u8, in_=mask_r[jt])
                m8 = mpool.tile([128, ROWS], u8, tag="m8")
                for b in range(8):
                    nc.vector.tensor_scalar(
                        out=m8[:, b * KB:(b + 1) * KB], in0=p_u8,
                        scalar1=b, scalar2=1,
                        op0=OP.logical_shift_right, op1=OP.bitwise_and)
                z_t = zpool.tile([128, ROWS], f16, tag="z")
                nc.vector.scalar_tensor_tensor(
                    out=z_t, in0=s_src_bc, scalar=sdc[:, jt:jt + 1],
                    in1=m8, op0=OP.add, op1=OP.mult)
                nc.scalar.activation(out=z_t, in_=z_t, func=AF.Prelu,
                                     alpha=ALPHA)
                p_t = ppool.tile([128, ROWS], f16, tag="p")
                nc.scalar.activation(out=p_t, in_=z_t, func=AF.Exp)
                for m in range(N_IT):
                    nc.tensor.matmul(
                        out=hh[m], lhsT=p_t[:, m * 128:(m + 1) * 128],
                        rhs=h_aug[:, jt, :D_OUT + 1],
                        start=(jt == 0), stop=(jt == N_JT - 1))

            # ------------- epilogue: out = elu(hh[:, :128] / Z) -------------
            for m in range(N_IT):
                rz = sm.tile([128, 1], f32, tag="rz")
                nc.vector.reciprocal(out=rz, in_=hh[m][:, D_OUT:D_OUT + 1])
                tmin = sm.tile([128, D_OUT], f32, tag="tmin")
                nc.vector.tensor_scalar_min(tmin, hh[m][:, :D_OUT], 0.0)
                wmax = sm.tile([128, D_OUT], f32, tag="wmax")
                nc.vector.tensor_scalar(
                    out=wmax, in0=hh[m][:, :D_OUT], scalar1=0.0, scalar2=rz,
                    op0=OP.max, op1=OP.mult)
                e_t = sm.tile([128, D_OUT], f32, tag="et")
                nc.scalar.activation(out=e_t, in_=tmin, func=AF.Exp, scale=rz)
                o_t = sm.tile([128, D_OUT], f16, tag="ot")
                nc.vector.scalar_tensor_tensor(
                    out=o_t, in0=e_t, scalar=-1.0, in1=wmax,
                    op0=OP.add, op1=OP.add)
                # rows i = 8q + m  (undo the bit-plane permutation)
                DMA(out=bass.AP(tensor=out_ap.tensor, offset=D_OUT * m,
                                ap=[[8 * D_OUT, 128], [1, D_OUT]]),
                    in_=o_t)
            hh_ps_cm.__exit__(None, None, None)

    nc.compile()
    return nc


def _get_nc():
    if "nc" not in _BUILT:
        _BUILT["nc"] = _build_nc()
    return _BUILT["nc"]


_last_exec_ns = None


def _config_jax_cache():
    if "cache" in _BUILT:
        return
    _BUILT["cache"] = True
    try:
        import jax

        jax.config.update("jax_compilation_cache_dir", "/tmp/gat_jax_cache")
        jax.config.update("jax_persistent_cache_min_compile_time_secs", 0.0)
        jax.config.update("jax_persistent_cache_min_entry_size_bytes", 0)
    except Exception:
        pass


def _get_prep():
    """Fused host prep on XLA-CPU: one pass packs the adjacency bits and
    assembles the fp16 [x_t | w | att] combo (~4x faster than numpy)."""
    if "prep" in _BUILT:
        return _BUILT["prep"]
    import functools

    import jax
    import jax.numpy as jnp

    @functools.partial(jax.jit, backend="cpu")
    def prep(nbr, x, w, att):
        y = (nbr > 0).astype(jnp.uint8).reshape(N // 8, 8, N)
        acc = y[:, 0, :]
        for b in range(1, 8):
            acc = acc | (y[:, b, :] << b)
        xt = x.astype(jnp.float16).reshape(
            N_CORES, ROWS, D_IN).transpose(0, 2, 1)
        wb = jnp.broadcast_to(
            w.astype(jnp.float16)[None], (N_CORES, D_IN, D_OUT))
        top = jnp.concatenate([xt, wb], axis=2)
        attrow = jnp.zeros((N_CORES, 1, CW), jnp.float16)
        attrow = attrow.at[:, 0, :2 * D_OUT].set(
            att.astype(jnp.float16)[None])
        combo = jnp.concatenate([top, attrow], axis=1)
        return acc, combo

    _BUILT["prep"] = prep
    return prep


def kernel(x, immediate_neighbor, weights, attention):
    import os

    _config_jax_cache()
    from concourse.bass_utils import run_bass_kernel_spmd

    x = np.asarray(x, dtype=np.float32)
    nbr = np.asarray(immediate_neighbor)
    w = np.asarray(weights, dtype=np.float32)
    att = np.asarray(attention, dtype=np.float32).reshape(2 * D_OUT)

    # prepack[k, j] bit b = (nbr[8k+b, j] > 0)  (== packbits(nbr > 0,
    # axis=0, bitorder='little')); combo = [x_t | w] rows + att row.
    # Both from one fused XLA-CPU jit: single pass over nbr, hw f16
    # conversion (~4x faster than the numpy equivalent).
    prepack_j, combo_j = _get_prep()(nbr, x, w, att)
    prepack = np.asarray(prepack_j)   # zero-copy on CPU backend
    combo = np.asarray(combo_j)

    nc = _get_nc()
    in_maps = []
    for c in range(N_CORES):
        in_maps.append({
            "combo": combo[c],
            # transposed view: run_bass_via_pjrt's np.concatenate does the
            # single gather copy, so no ascontiguousarray here
            "maskp": prepack[c * KB:(c + 1) * KB, :].T,
        })
    kw = {}
    if os.environ.get("GAT_TRACE"):
        kw["trace"] = True
        tdir = os.environ.get("GAT_TRACE_DIR", "/tmp/gat_trace")
        os.makedirs(tdir, exist_ok=True)
        kw["tmpdir"] = tdir
    res = run_bass_kernel_spmd(nc, in_maps, list(range(N_CORES)), **kw)
    global _last_exec_ns
    _last_exec_ns = res.exec_time_ns
    out = np.empty((N, D_OUT), np.float32)
    for c in range(N_CORES):
        out[c * ROWS:(c + 1) * ROWS] = res.results[c]["out"]
    return out


# revision 24
# speedup vs baseline: 1.2570x; 1.0848x over previous
"""GAT layer (nn_GATLayer) as a Bass/Tile SPMD kernel on 8 trn2 NeuronCores.

Row-sharded: core c owns output rows [c*1024, (c+1)*1024).
  h = x @ W                       (local block + AllGather, fp16)
  e = leaky_relu(s_src[i] + s_dst[j]), s_* = h @ a_*
  masked = where(nbr>0, e, 0) == leaky_relu(nbr * (s_src[i]+s_dst[j]))
  att = softmax(masked, axis=1)   (no max-subtraction needed: |z| small)
  out = elu(att @ h)

Wall-clock (axon tunnel) optimizations vs v1:
  - adjacency shipped BITPACKED (u8, 32x fewer bytes; unpacked on DVE
    with shift+and per bit-plane)
  - x/w/att shipped as one fp16 tensor (fewer device_put round trips)
  - compute runs TRANSPOSED (partition=j, free=i): the attention matrix
    is produced directly in lhsT layout, so no PE transposes, no PSUM
    staging, no identity matrix. The bit-unpack column permutation
    (c = b*128+k <-> i = 8k+b) lands on the output-row side and is
    undone by a strided output DMA.
  - fp16 output, jax persistent compilation cache
"""

import sys

for _p in ("/opt/trn_rl_repo",):
    if _p not in sys.path:
        sys.path.insert(0, _p)

import numpy as np

N_CORES = 8
N = 8192               # nodes
D_IN = 512             # input features
D_OUT = 128            # output features
ROWS = N // N_CORES    # rows per core (1024)
N_IT = ROWS // 128     # i-subtiles per core (8)
N_JT = N // 128        # j-tiles (64)
HCOL = 132             # h row: 128 features + 1.0 + padding
KB = ROWS // 8         # packed mask bytes per row (128)
CW = 1152              # combo width: 1024 (x_t) + 128 (w)
ALPHA = 0.2

_BUILT = {}


def _build_nc():
    import concourse.bacc as bacc
    import concourse.bass as bass
    import concourse.tile as tile
    from concourse import mybir

    f32 = mybir.dt.float32
    f16 = mybir.dt.float16
    u8 = mybir.dt.uint8
    AF = mybir.ActivationFunctionType
    OP = mybir.AluOpType

    nc = bacc.Bacc("TRN2", target_bir_lowering=False, debug=False,
                   num_devices=N_CORES)
    DMA = nc.sync.dma_start

    # combo rows 0..511: [x_t strip | w]; row 512: att (256 cols)
    combo_in = nc.declare_dram_parameter("combo", [D_IN + 1, CW], f16,
                                         isOutput=False)
    # maskp[j, k] bit b  =  (nbr[i_local=8k+b, j] > 0)
    mask_in = nc.declare_dram_parameter("maskp", [N, KB], u8, isOutput=False)
    out_d = nc.declare_dram_parameter("out", [ROWS, D_OUT], f16, isOutput=True)

    mask_r = mask_in[:, :].rearrange("(t p) k -> t p k", p=128)
    combo_ap = combo_in[:, :]
    out_ap = out_d[:, :]
    out_r = out_d[:, :].rearrange("(t p) n -> t p n", p=128)

    import os as _os
    if _os.environ.get("GAT_STOP"):
        with tile.TileContext(nc) as tc:
            with tc.tile_pool(name="sb", bufs=1) as sb:
                zt = sb.tile([128, D_OUT], f16)
                nc.vector.memset(zt, 0.0)
                for it in range(N_IT):
                    DMA(out=out_r[it], in_=zt)
        nc.compile()
        return nc

    with tile.TileContext(nc) as tc:
        with (
            tc.tile_pool(name="const", bufs=1) as const,
            tc.tile_pool(name="dram", bufs=1, space="DRAM") as dram,
            tc.tile_pool(name="mpool", bufs=3) as mpool,
            tc.tile_pool(name="zpool", bufs=3) as zpool,
            tc.tile_pool(name="ppool", bufs=3) as ppool,
            tc.tile_pool(name="sm", bufs=2) as sm,
        ):
            s_src_bc = const.tile([128, ROWS], f16)   # permuted: col b*KB+k -> i=8k+b
            sdc = const.tile([128, N_JT], f32)        # sdc[p, t] = s_dst[128t + p]
            h_aug = const.tile([128, N_JT, HCOL], f16)

            h16_loc = dram.tile([ROWS, HCOL], f16)
            h16_full = dram.tile([N, HCOL], f16)
            ssrc_loc = dram.tile([1, ROWS], f16)
            sd_loc = dram.tile([1, ROWS], f32)
            sd_full = dram.tile([1, N], f32)

            with (
                tc.tile_pool(name="pre_sb", bufs=1) as pre_sb,
                tc.tile_pool(name="pre_ps", bufs=2, space="PSUM") as pre_ps,
            ):
                att_row = pre_sb.tile([1, 2 * D_OUT], f16)
                DMA(out=att_row, in_=combo_in[D_IN:D_IN + 1, 0:2 * D_OUT])
                ones_1 = pre_sb.tile([1, 128], f16)
                nc.vector.memset(ones_1, 1.0)
                att_ps = pre_ps.tile([128, 2 * D_OUT], f32, tag="pp")
                nc.tensor.matmul(out=att_ps, lhsT=ones_1, rhs=att_row,
                                 start=True, stop=True)
                att_bc = pre_sb.tile([128, 2 * D_OUT], f32)
                nc.scalar.copy(out=att_bc, in_=att_ps)

                # w: combo[0:512, 1024:1152] -> [p, t, n], d_in = 128t + p
                w_sb = pre_sb.tile([128, 4, D_OUT], f16)
                DMA(out=w_sb,
                    in_=bass.AP(tensor=combo_ap.tensor, offset=1024,
                                ap=[[CW, 128], [128 * CW, 4], [1, D_OUT]]))
                # x_t: combo[0:512, 0:1024] -> [p, t, s, q], d=128t+p, i=128s+q
                xt_sb = pre_sb.tile([128, 4, N_IT, 128], f16)
                DMA(out=xt_sb,
                    in_=bass.AP(tensor=combo_ap.tensor, offset=0,
                                ap=[[CW, 128], [128 * CW, 4], [128, N_IT],
                                    [1, 128]]))

                h16_sb = pre_sb.tile([128, N_IT, HCOL], f16)
                nc.vector.memset(h16_sb[:, :, D_OUT:], 0.0)
                nc.gpsimd.memset(h16_sb[:, :, D_OUT:D_OUT + 1], 1.0)
                s_src_sb = pre_sb.tile([128, N_IT], f32)
                s_dst_sb = pre_sb.tile([128, N_IT], f32)
                scrap = pre_sb.tile([128, 128], f32)
                scrap2 = pre_sb.tile([128, 128], f32)
                for s in range(N_IT):
                    h_ps = pre_ps.tile([128, D_OUT], f32, tag="pp")
                    for t in range(4):
                        nc.tensor.matmul(out=h_ps, lhsT=xt_sb[:, t, s, :],
                                         rhs=w_sb[:, t, :],
                                         start=(t == 0), stop=(t == 3))
                    nc.vector.tensor_mul(scrap, h_ps, att_bc[:, :D_OUT])
                    nc.vector.tensor_reduce(
                        out=s_src_sb[:, s:s + 1], in_=scrap,
                        axis=mybir.AxisListType.X, op=OP.add)
                    nc.vector.tensor_mul(scrap2, h_ps, att_bc[:, D_OUT:])
                    nc.vector.tensor_reduce(
                        out=s_dst_sb[:, s:s + 1], in_=scrap2,
                        axis=mybir.AxisListType.X, op=OP.add)
                    nc.scalar.copy(out=h16_sb[:, s, :D_OUT], in_=h_ps)

                ssrc16 = pre_sb.tile([128, N_IT], f16)
                nc.vector.tensor_copy(out=ssrc16, in_=s_src_sb)

                # flatten to DRAM: flat[128s + p] = value[p, s]
                DMA(out=bass.AP(tensor=ssrc_loc[:, :].tensor, offset=0,
                                ap=[[1, 128], [128, N_IT]]),
                    in_=ssrc16)
                DMA(out=bass.AP(tensor=sd_loc[:, :].tensor, offset=0,
                                ap=[[1, 128], [128, N_IT]]),
                    in_=s_dst_sb)
                DMA(out=h16_loc[:, :].rearrange("(s p) c -> p s c", p=128),
                    in_=h16_sb)

                nc.gpsimd.collective_compute(
                    "AllGather", OP.bypass,
                    replica_groups=[list(range(N_CORES))],
                    ins=[h16_loc[:, :].opt()], outs=[h16_full[:, :].opt()])
                nc.gpsimd.collective_compute(
                    "AllGather", OP.bypass,
                    replica_groups=[list(range(N_CORES))],
                    ins=[sd_loc[:, :].opt()], outs=[sd_full[:, :].opt()])

                DMA(out=h_aug,
                    in_=h16_full[:, :].rearrange("(t p) c -> p t c", p=128))
                DMA(out=sdc,
                    in_=bass.AP(tensor=sd_full[:, :].tensor, offset=0,
                                ap=[[1, 128], [128, N_JT]]))
                # s_src broadcast, permuted: col b*KB+k -> s_src[8k+b]
                for b in range(8):
                    DMA(out=s_src_bc[:, b * KB:(b + 1) * KB],
                        in_=bass.AP(tensor=ssrc_loc[:, :].tensor, offset=b,
                                    ap=[[0, 128], [8, KB]]))

            # one PSUM bank per accumulator (a start=True matmul resets the
            # whole bank, so accumulator groups must not share banks);
            # opened only after pre_ps closes so all 8 banks are free
            hh_ps_cm = tc.tile_pool(name="hh_ps", bufs=1, space="PSUM")
            hh_ps = hh_ps_cm.__enter__()
            hh = []
            for m in range(N_IT):
                hh_m = hh_ps.tile([128, D_OUT + 1], f32, tag=f"hh{m}",
                                  name=f"hh{m}")
                hh.append(hh_m)

            # ---------------- main loop over j-tiles ----------------
            for jt in range(N_JT):
                p_u8 = mpool.tile([128, KB], u8, tag="pk")
                DMA(out=p_u8, in_=mask_r[jt])
                m8 = mpool.tile([128, ROWS], u8, tag="m8")
                for b in range(8):
                    nc.vector.tensor_scalar(
                        out=m8[:, b * KB:(b + 1) * KB], in0=p_u8,
                        scalar1=b, scalar2=1,
                        op0=OP.logical_shift_right, op1=OP.bitwise_and)
                z_t = zpool.tile([128, ROWS], f16, tag="z")
                nc.vector.scalar_tensor_tensor(
                    out=z_t, in0=s_src_bc, scalar=sdc[:, jt:jt + 1],
                    in1=m8, op0=OP.add, op1=OP.mult)
                nc.scalar.activation(out=z_t, in_=z_t, func=AF.Prelu,
                                     alpha=ALPHA)
                p_t = ppool.tile([128, ROWS], f16, tag="p")
                nc.scalar.activation(out=p_t, in_=z_t, func=AF.Exp)
                for m in range(N_IT):
                    nc.tensor.matmul(
                        out=hh[m], lhsT=p_t[:, m * 128:(m + 1) * 128],
                        rhs=h_aug[:, jt, :D_OUT + 1],
                        start=(jt == 0), stop=(jt == N_JT - 1))

            # ------------- epilogue: out = elu(hh[:, :128] / Z) -------------
            for m in range(N_IT):
                rz = sm.tile([128, 1], f32, tag="rz")
                nc.vector.reciprocal(out=rz, in_=hh[m][:, D_OUT:D_OUT + 1])
                tmin = sm.tile([128, D_OUT], f32, tag="tmin")
                nc.vector.tensor_scalar_min(tmin, hh[m][:, :D_OUT], 0.0)
                wmax = sm.tile([128, D_OUT], f32, tag="wmax")
                nc.vector.tensor_scalar(
                    out=wmax, in0=hh[m][:, :D_OUT], scalar1=0.0, scalar2=rz,
                    op0=OP.max, op1=OP.mult)
                e_t = sm.tile([128, D_OUT], f32, tag="et")
                nc.scalar.activation(out=e_t, in_=tmin, func=AF.Exp, scale=rz)
                o_t = sm.tile([128, D_OUT], f16, tag="ot")
                nc.vector.scalar_tensor_tensor(
                    out=o_t, in0=e_t, scalar=-1.0, in1=wmax,
                    op0=OP.add, op1=OP.add)
                # rows i = 8q + m  (undo the bit-plane permutation)
                DMA(out=bass.AP(tensor=out_ap.tensor, offset=D_OUT * m,
                                ap=[[8 * D_OUT, 128], [1, D_OUT]]),
                    in_=o_t)
            hh_ps_cm.__exit__(None, None, None)

    nc.compile()
    return nc


def _get_nc():
    if "nc" not in _BUILT:
        _BUILT["nc"] = _build_nc()
    return _BUILT["nc"]


_last_exec_ns = None


def _config_jax_cache():
    if "cache" in _BUILT:
        return
    _BUILT["cache"] = True
    try:
        import jax

        jax.config.update("jax_compilation_cache_dir", "/tmp/gat_jax_cache")
        jax.config.update("jax_persistent_cache_min_compile_time_secs", 0.0)
        jax.config.update("jax_persistent_cache_min_entry_size_bytes", 0)
    except Exception:
        pass


def _get_prep():
    """Fused host prep on XLA-CPU: one pass packs the adjacency bits and
    assembles the fp16 [x_t | w | att] combo (~4x faster than numpy)."""
    if "prep" in _BUILT:
        return _BUILT["prep"]
    import functools

    import jax
    import jax.numpy as jnp

    @functools.partial(jax.jit, backend="cpu")
    def prep(nbr, x, w, att):
        y = (nbr > 0).astype(jnp.uint8).reshape(N // 8, 8, N)
        acc = y[:, 0, :]
        for b in range(1, 8):
            acc = acc | (y[:, b, :] << b)
        # core-major transposed strips [8, N, KB] so the downstream
        # per-core concat copies contiguous blocks
        mT = acc.reshape(N_CORES, KB, N).transpose(0, 2, 1)
        xt = x.astype(jnp.float16).reshape(
            N_CORES, ROWS, D_IN).transpose(0, 2, 1)
        wb = jnp.broadcast_to(
            w.astype(jnp.float16)[None], (N_CORES, D_IN, D_OUT))
        top = jnp.concatenate([xt, wb], axis=2)
        attrow = jnp.zeros((N_CORES, 1, CW), jnp.float16)
        attrow = attrow.at[:, 0, :2 * D_OUT].set(
            att.astype(jnp.float16)[None])
        combo = jnp.concatenate([top, attrow], axis=1)
        return mT, combo

    _BUILT["prep"] = prep
    return prep


def kernel(x, immediate_neighbor, weights, attention):
    import os

    _config_jax_cache()
    from concourse.bass_utils import run_bass_kernel_spmd

    x = np.asarray(x, dtype=np.float32)
    nbr = np.asarray(immediate_neighbor)
    w = np.asarray(weights, dtype=np.float32)
    att = np.asarray(attention, dtype=np.float32).reshape(2 * D_OUT)

    # prepack[k, j] bit b = (nbr[8k+b, j] > 0)  (== packbits(nbr > 0,
    # axis=0, bitorder='little')); combo = [x_t | w] rows + att row.
    # Both from one fused XLA-CPU jit: single pass over nbr, hw f16
    # conversion (~4x faster than the numpy equivalent).
    prepack_j, combo_j = _get_prep()(nbr, x, w, att)
    prepack = np.asarray(prepack_j)   # zero-copy on CPU backend
    combo = np.asarray(combo_j)

    nc = _get_nc()
    in_maps = []
    for c in range(N_CORES):
        in_maps.append({
            "combo": combo[c],
            "maskp": prepack[c],
        })
    kw = {}
    if os.environ.get("GAT_TRACE"):
        kw["trace"] = True
        tdir = os.environ.get("GAT_TRACE_DIR", "/tmp/gat_trace")
        os.makedirs(tdir, exist_ok=True)
        kw["tmpdir"] = tdir
    res = run_bass_kernel_spmd(nc, in_maps, list(range(N_CORES)), **kw)
    global _last_exec_ns
    _last_exec_ns = res.exec_time_ns
    out = np.empty((N, D_OUT), np.float32)
    for c in range(N_CORES):
        out[c * ROWS:(c + 1) * ROWS] = res.results[c]["out"]
    return out
